# revision 9
# baseline (speedup 1.0000x reference)
import base64
import hashlib
import io
import time as _time

import numpy as np

# nn_GraphTransformerDemon: B=4, S=384, IN=32, H=64, NH=4
#
# Split: the transformer encoder (O(B*S*H^2), tiny) runs on host; the
# O(B*S^2*H) all-pairs edge/demon grid runs on the 8 NeuronCores, one
# (batch b, i-half) pair per core.  Only pooled sums are needed:
#   SA[h]  = sum_ij relu(L_i + R_j)[h]
#   SAK[h] = sum_ij keep_ij * relu(L_i + R_j)[h]
#   SK     = sum_ij keep_ij
# with keep_ij = sigmoid(sum_h' Wd2[h'] * relu(dL_i + dR_j)[h'] + bd2).
# msgs@We2 is folded algebraically on the host afterwards.
#
# Per-call device I/O is minimized: the only ExternalInput is nodes[b].T
# (the L/R/dL/dR projections are computed on-device); the edge/demon MLP
# weights and the identity patterns are Const tensors baked into the NEFF
# (build cache keyed on the weight bytes, so changed weights recompile).
# The executor (jit of the bass_exec custom call) is built once and
# cached; per-call cost is one host->device transfer of 8 x [64,384] f32,
# one NEFF execution, and one blocking fetch of 8 x [2,80] f32.

B, S, IN, H, NH, DH, NC_ = 4, 384, 32, 64, 4, 16, 3
ISH = S // 2            # 192 i-rows per core
NJT = S // 128          # 3 j-tiles
NCE = ISH // 8          # 24 eh chunks (8 i x 64 h = 512)
NCD = ISH // 16         # 12 dh chunks (16 i x 32 h = 512)

_BUILT = {}


def _inline_const(nc, data, dtype, name):
    """inline_tensor with an explicit BIR dtype (e.g. float32r)."""
    from concourse.bass import DRamTensorHandle

    data = np.ascontiguousarray(data.astype(np.float32))
    mls = nc._tensor(name, list(data.shape), dtype, kind="Const", type="DRAM")
    buf = io.BytesIO()
    np.save(buf, data, allow_pickle=False)
    mls.file = f"{name}.npy"
    mls.ant_data = base64.standard_b64encode(buf.getvalue()).decode()
    return DRamTensorHandle(name, list(data.shape), dtype)


def _build(We1, be1, Wd1, bd1, Wd2, bd2f):
    import concourse.bass as bass
    import concourse.bacc as bacc
    import concourse.mybir as mybir
    from concourse import tile

    f32 = mybir.dt.float32
    f32r = mybir.dt.float32r
    bf16 = mybir.dt.bfloat16
    AF = mybir.ActivationFunctionType
    AL = mybir.AluOpType
    AX = mybir.AxisListType

    nc = bacc.Bacc("TRN2", target_bir_lowering=False, debug=False, num_devices=8)
    nodt_d = nc.dram_tensor("nodt", [64, S], bf16, kind="ExternalInput")
    out_d = nc.dram_tensor("out", [1, 132], f32, kind="ExternalOutput")

    # weights packed for the two projection matmuls (contraction K=65:
    # 64 hidden + 1 ones row carrying the biases)
    w1p = np.zeros((65, 192), np.float32)
    w1p[:64, 0:64] = We1[:H]        # L
    w1p[:64, 64:96] = Wd1[:H]       # dL
    w1p[:64, 96:160] = We1[H:]      # R
    w1p[:64, 160:192] = Wd1[H:]     # dR
    w1p[64, 0:64] = be1
    w1p[64, 64:96] = bd1
    w1p_d = _inline_const(nc, w1p, f32r, "w1p")
    idente_d = _inline_const(
        nc, np.tile(np.eye(64, dtype=np.float32), (1, NCE * 8)), f32r, "idente")
    identd_d = _inline_const(
        nc, np.tile(np.eye(32, dtype=np.float32), (1, NCD * 16)), f32r, "identd")
    wd2_d = _inline_const(
        nc, np.tile(Wd2[:, 0].astype(np.float32), (128, 16)), f32, "wd2rep")
    ones_d = _inline_const(nc, np.ones((128, 2 * ISH), np.float32), f32r, "onesr")
    ones1_d = _inline_const(nc, np.ones((128, 1), np.float32), f32, "onesf")
    sel16_d = _inline_const(nc, np.eye(16, dtype=np.float32), f32, "sel16")

    with tile.TileContext(nc) as tc:
        with (
            tc.tile_pool(name="const", bufs=1) as cp,
            tc.tile_pool(name="work", bufs=4) as wp,
            tc.tile_pool(name="psg", bufs=2, space="PSUM") as psg,
            tc.tile_pool(name="pse", bufs=3, space="PSUM") as pse,
            tc.tile_pool(name="psr", bufs=1, space="PSUM") as psr,
        ):
            w1sb = cp.tile([65, 192], f32r)
            nc.sync.dma_start(w1sb[:], w1p_d[:])
            wd2 = cp.tile([128, 512], f32)
            nc.sync.dma_start(wd2[:], wd2_d[:])
            nod16 = cp.tile([64, S], bf16)
            nc.sync.dma_start(nod16[:], nodt_d[:])
            nodtp = cp.tile([65, S], f32r)
            nc.vector.tensor_copy(nodtp[0:64, :], nod16[:])
            nc.sync.dma_start(nodtp[64:65, :], ones_d[0:1, 0:S])

            rhs_eh = cp.tile([H + 1, NCE * 512], f32r)
            nc.sync.dma_start(rhs_eh[0:64, :], idente_d[:])
            rhs_dh = cp.tile([33, NCD * 512], f32r)
            nc.sync.dma_start(rhs_dh[0:32, :], identd_d[:])

            # on-device projections: A = [R.T; dR.T] over all S columns,
            # B = [L.T; dL.T] over this core's i-half (host pre-rolls the
            # nodes so the local half is always columns 0:ISH)
            ps_a = pse.tile([96, S], f32, tag="pe", name="psa")
            nc.tensor.matmul(ps_a[:], w1sb[:, 96:192], nodtp[:], start=True, stop=True)
            # L/dL in (i, h) layout directly: lhsT = nodes slice, rhs = weights
            # (the ones row of nodtp picks up the bias row of w1sb)
            ps_b = pse.tile([96, 192], f32, tag="pe", name="psb")
            nc.tensor.matmul(ps_b[:, 0:96], nodtp[:, 0:96], w1sb[:, 0:96],
                             start=True, stop=True)
            nc.tensor.matmul(ps_b[:, 96:192], nodtp[:, 96:192], w1sb[:, 0:96],
                             start=True, stop=True)

            rt = cp.tile([H + 1, S], f32r)
            nc.vector.tensor_copy(rt[0:64, :], ps_a[0:64, :])
            nc.sync.dma_start(rt[64:65, :], ones_d[0:1, 0:S])
            drt = cp.tile([33, S], f32r)
            nc.vector.tensor_copy(drt[0:32, :], ps_a[64:96, :])
            nc.sync.dma_start(drt[32:33, :], ones_d[0:1, 0:S])

            # ld2[i_local, :]: cols 0:64 L / 64:96 dL (i-chunk0), 96:160 L /
            # 160:192 dL (i-chunk1); flatten to the single-row lf/dlf layout
            # (i major, h minor) with contiguous-destination DMAs
            ld2 = cp.tile([96, 192], f32r)
            nc.vector.tensor_copy(ld2[:], ps_b[:])
            nc.sync.dma_start(rhs_eh[64:65, 0:96 * 64], ld2[0:96, 0:64])
            nc.sync.dma_start(rhs_eh[64:65, 96 * 64:192 * 64], ld2[0:96, 96:160])
            nc.sync.dma_start(rhs_dh[32:33, 0:96 * 32], ld2[0:96, 64:96])
            nc.sync.dma_start(rhs_dh[32:33, 96 * 32:192 * 32], ld2[0:96, 160:192])

            keep = [cp.tile([128, 2 * ISH], f32r, tag=f"keep{j}", name=f"keep{j}")
                    for j in range(NJT)]
            klog = [cp.tile([128, ISH], f32, tag=f"klog{j}", name=f"klog{j}")
                    for j in range(NJT)]
            for jt in range(NJT):
                nc.sync.dma_start(keep[jt][:], ones_d[:])
            ksum = cp.tile([128, 4], f32)
            nc.gpsimd.memset(ksum[:], 0.0)
            red_ps = psr.tile([16, 512], f32)

            def dh_chunk(jt, c):
                pd = psg.tile([128, 512], f32, tag="pd", name="pd")
                nc.tensor.matmul(
                    pd[:], drt[:, jt * 128:(jt + 1) * 128],
                    rhs_dh[:, c * 512:(c + 1) * 512],
                    start=True, stop=True)
                dhw = wp.tile([128, 512], f32, tag="dhw", name="dhw")
                nc.vector.scalar_tensor_tensor(
                    out=dhw[:], in0=pd[:], scalar=0.0, in1=wd2[:],
                    op0=AL.max, op1=AL.mult)
                nc.vector.tensor_reduce(
                    out=klog[jt][:, c * 16:(c + 1) * 16],
                    in_=dhw[:].rearrange("p (i h) -> p i h", h=32),
                    axis=AX.X, op=AL.add)

            def sigmoid_keep(jt):
                kview = keep[jt][:].rearrange("p (i two) -> p two i", two=2)
                nc.scalar.activation(
                    kview[:, 0, :], klog[jt][:], AF.Sigmoid, bias=float(bd2f))
                nc.vector.tensor_reduce(
                    out=ksum[:, jt:jt + 1], in_=kview[:, 0, :],
                    axis=AX.X, op=AL.add)

            nmm = NJT * NCE
            mm = 0
            # software pipeline: reduce-matmuls trail the gen matmul by one
            # chunk so PE never stalls on ACT's relu; dh-gen for jt+1 is
            # interleaved into the eh phase of jt so DVE keep-path work
            # overlaps ACT/PE eh work.
            for c in range(NCD):
                dh_chunk(0, c)
            for jt in range(NJT):
                sigmoid_keep(jt)
                pend = None
                for c in range(NCE):
                    pe_ = pse.tile([128, 512], f32, tag="pe", name="pe")
                    nc.tensor.matmul(
                        pe_[:], rt[:, jt * 128:(jt + 1) * 128],
                        rhs_eh[:, c * 512:(c + 1) * 512],
                        start=True, stop=True)
                    eh = wp.tile([128, 512], f32r, tag="eh", name="eh")
                    nc.scalar.activation(eh[:], pe_[:], AF.Relu)
                    if jt + 1 < NJT and c % 2 == 0:
                        dh_chunk(jt + 1, c // 2)
                    if pend is not None:
                        pc, peh = pend
                        nc.tensor.matmul(
                            red_ps[:], keep[jt][:, pc * 16:(pc + 1) * 16],
                            peh[:], start=(mm == 0), stop=(mm == nmm - 1))
                        mm += 1
                    pend = (c, eh)
                pc, peh = pend
                nc.tensor.matmul(
                    red_ps[:], keep[jt][:, pc * 16:(pc + 1) * 16],
                    peh[:], start=(mm == 0), stop=(mm == nmm - 1))
                mm += 1

            red_sb = cp.tile([16, 512], f32)
            nc.vector.tensor_copy(red_sb[:], red_ps[:])

            # compact to [1, 132] = SAK(64) | SA(64) | SK(4) with PE
            # selector matmuls (partition-dim reductions):
            #   SAK[h] = sum_g red[2g, 64g+h], SA[h] = sum_g red[1, 64g+h],
            #   SK = sum_p ksum[p, 0:4]
            sel = cp.tile([16, 16], f32)
            nc.sync.dma_start(sel[:], sel16_d[:])
            onescol = cp.tile([128, 1], f32)
            nc.sync.dma_start(onescol[:], ones1_d[:])
            pc2 = psg.tile([1, 132], f32, tag="pd", name="pc2")
            for g in range(8):
                nc.tensor.matmul(
                    pc2[:, 0:64], sel[:, 2 * g:2 * g + 1],
                    red_sb[:, 64 * g:64 * (g + 1)], start=(g == 0), stop=(g == 7))
            for g in range(8):
                nc.tensor.matmul(
                    pc2[:, 64:128], sel[:, 1:2],
                    red_sb[:, 64 * g:64 * (g + 1)], start=(g == 0), stop=(g == 7))
            nc.tensor.matmul(pc2[:, 128:132], onescol[:], ksum[:],
                             start=True, stop=True)
            acc = cp.tile([1, 132], f32)
            nc.vector.tensor_copy(acc[:], pc2[:])
            nc.sync.dma_start(out_d[:], acc[:])
    nc.compile()
    return nc


def _host_nodes(d):
    x = d["x"]
    n = np.maximum(x @ d["Wp"] + d["bp"], 0.0)
    q = (n @ d["Wq"] + d["bq"]).reshape(B, S, NH, DH)
    k = (n @ d["Wk"] + d["bk"]).reshape(B, S, NH, DH)
    v = (n @ d["Wv"] + d["bv"]).reshape(B, S, NH, DH)
    sc = np.einsum("bqhd,bkhd->bhqk", q, k) / np.float32(np.sqrt(DH))
    sc = sc - sc.max(-1, keepdims=True)
    e = np.exp(sc)
    a = e / e.sum(-1, keepdims=True)
    att = np.einsum("bhqk,bkhd->bqhd", a, v).reshape(B, S, H) @ d["Wo"] + d["bo"]

    def ln(t, g, b):
        m = t.mean(-1, keepdims=True)
        vv = ((t - m) ** 2).mean(-1, keepdims=True)
        return (t - m) / np.sqrt(vv + np.float32(1e-5)) * g + b

    n = ln(n + att, d["g1"], d["b1"])
    ff = np.maximum(n @ d["Wf1"] + d["bf1"], 0.0) @ d["Wf2"] + d["bf2"]
    return ln(n + ff, d["g2"], d["b2"]).astype(np.float32)


def _make_exec(nc):
    """Once-compiled jit of the bass_exec custom call across the 8 cores
    (same lowering as bass2jax.run_bass_via_pjrt, but cached so steady-state
    calls skip re-trace / re-compile)."""
    import jax
    from jax.experimental.shard_map import shard_map
    from jax.sharding import Mesh, PartitionSpec

    from concourse import mybir
    from concourse.bass2jax import (_bass_exec_p, install_neuronx_cc_hook,
                                    partition_id_tensor)

    install_neuronx_cc_hook()
    pname = nc.partition_id_tensor.name if nc.partition_id_tensor else None
    in_names, out_names, out_avals = [], [], []
    for alloc in nc.m.functions[0].allocations:
        if not isinstance(alloc, mybir.MemoryLocationSet):
            continue
        name = alloc.memorylocations[0].name
        if alloc.kind == "ExternalInput":
            if name != pname:
                in_names.append(name)
        elif alloc.kind == "ExternalOutput":
            out_names.append(name)
            out_avals.append(jax.core.ShapedArray(
                tuple(alloc.tensor_shape), mybir.dt.np(alloc.dtype)))
    n_params, n_outs = len(in_names), len(out_names)
    all_names = in_names + out_names + ([pname] if pname else [])
    donate = tuple(range(n_params, n_params + n_outs))

    def _body(*args):
        ops = list(args)
        if pname:
            ops.append(partition_id_tensor())
        return tuple(_bass_exec_p.bind(
            *ops,
            out_avals=tuple(out_avals),
            in_names=tuple(all_names),
            out_names=tuple(out_names),
            lowering_input_output_aliases=(),
            sim_require_finite=True,
            sim_require_nnan=True,
            nc=nc,
        ))

    mesh = Mesh(np.asarray(jax.devices()[:8]), ("core",))
    P = PartitionSpec("core")
    sharded = jax.jit(
        shard_map(_body, mesh=mesh, in_specs=(P,) * (n_params + n_outs),
                  out_specs=(P,) * n_outs, check_rep=False),
        donate_argnums=donate, keep_unused=True)
    return sharded, in_names, out_names, out_avals


def kernel(**inputs):
    d = {k: np.asarray(v, dtype=np.float32) for k, v in inputs.items()}
    nodes = _host_nodes(d)
    We2, be2 = d["We2"], d["be2"]

    key = hashlib.sha256(b"".join(
        np.ascontiguousarray(d[k]).tobytes()
        for k in ("We1", "be1", "Wd1", "bd1", "Wd2", "bd2"))).hexdigest()
    st = _BUILT.get(key)
    if st is None:
        st = {"nc": _build(d["We1"], d["be1"], d["Wd1"], d["bd1"],
                           d["Wd2"], float(d["bd2"][0]))}
        for k in list(_BUILT):
            if k != "dev_ns":
                del _BUILT[k]
        _BUILT[key] = st

    # per-core input: nodes[b].T in bf16, rolled so the core's i-half is
    # cols 0:ISH (j-order permutation is sum-invariant)
    import ml_dtypes
    nodt_all = np.empty((8 * 64, S), ml_dtypes.bfloat16)
    for core in range(8):
        b, ih = core // 2, core % 2
        nb = nodes[b] if ih == 0 else np.roll(nodes[b], -ISH, axis=0)
        nodt_all[core * 64:(core + 1) * 64] = nb.T.astype(ml_dtypes.bfloat16)

    t0 = _time.perf_counter()
    if "exec" not in st:
        from concourse.bass_utils import run_bass_kernel_spmd
        in_maps = [{"nodt": np.ascontiguousarray(nodt_all[c * 64:(c + 1) * 64])}
                   for c in range(8)]
        r = run_bass_kernel_spmd(st["nc"], in_maps, list(range(8)))
        outs = np.stack([r.results[c]["out"] for c in range(8)])
        _BUILT["dev_ns"] = (_time.perf_counter() - t0) * 1e9
        st["exec"] = _make_exec(st["nc"])
        # warm the cached-executor jit (compiles on first call) and check it
        # agrees with the run_bass_kernel_spmd result
        sharded = st["exec"][0]
        warm = np.asarray(sharded(nodt_all, np.zeros((8 * 1, 132), np.float32))[0])
        assert np.allclose(warm.reshape(8, 1, 132), outs, rtol=1e-4, atol=1e-3), \
            "cached executor disagrees with run_bass_kernel_spmd"
    else:
        sharded, in_names, out_names, out_avals = st["exec"]
        zeros = np.zeros((8 * 1, 132), np.float32)
        res = sharded(nodt_all, zeros)
        outs = np.asarray(res[0]).reshape(8, 1, 132)
        _BUILT["dev_ns"] = (_time.perf_counter() - t0) * 1e9

    out = np.zeros((B, NC_), np.float32)
    for b in range(B):
        SA = np.zeros(H, np.float32)
        SAK = np.zeros(H, np.float32)
        SK = np.float32(0.0)
        for ih in range(2):
            r = outs[2 * b + ih][0]
            SAK += r[0:64]
            SA += r[64:128]
            SK += r[128:132].sum()
        pa = nodes[b].mean(0) + (SAK @ We2 + SK * be2) / np.float32(S)
        pt = ((SA - SAK) @ We2 + (np.float32(S * S) - SK) * be2) / np.float32(S)
        h = np.maximum(np.concatenate([pa, pt]) @ d["Wc1"] + d["bc1"], 0.0)
        out[b] = h @ d["Wc2"] + d["bc2"]
    return out.astype(np.float32)
